# revision 1
# baseline (speedup 1.0000x reference)
"""Self-contained Trainium2 Bass kernel for nn_Attention (additive attention scores).

kernel(**inputs) takes FULL unsharded inputs and returns the FULL output:
  decoder_hide [32, 512] f32, encoder_out [32, 2048, 1024] f32, mask [32, 2048] i32,
  W_attn [1536, 512] f32, b_attn [512] f32, v_w [512] f32  ->  out [32, 2048] f32

Strategy: data-parallel over batch across 8 NeuronCores (4 batches/core),
weights replicated. Per core (fp8 DoubleRow pipeline, ~131us main loop vs
~232-252us for the bf16 PE-transpose baseline; rel err ~9e-3 vs the 2e-2 gate):
  - encoder tiles loaded natural [s, e] with f32->bf16 cast during SWDGE DMA
    (measured fastest load path at ~103us, near the 89us/core HBM floor),
    then converted bf16->fp8e4 on the DVE -- round-to-nearest matters: the
    DMA's direct f32->fp8 cast truncates, doubling quantization noise and
    failing the accuracy gate
  - fp8 PAIRS viewed as 16-bit units and transposed on the TensorEngine
    (bit-identical bf16 view; Ldweights rejects integer dtypes): each
    [e-pair, s] tile's byte lanes are exactly the DoubleRow moving-operand
    interleave. DMA-xbar transposes were measured +86us (they serialize
    against loads on the shared SDMA engines); fp8 pairs halve the PE
    transpose count vs a bf16 layout
  - PE DoubleRow matmuls contract K=256 per call at 2 MACs/cell/cycle:
    256 projection matmuls instead of 512, each ~2x faster
  - W_e pre-scaled x16 before fp8 quantization (better e4m3 range use),
    compensated for free via the ACT tanh scale=1/16 parameter (ACT computes
    func(scale*in + bias), so the dec_proj bias goes in unscaled)
  - batch-outer loop order: each batch's load->convert->transpose->matmul->
    tanh->v-dot chain pipelines against its neighbors (an s-block-level
    barrier was measured ~10-25us slower)
  - v_w dot in bf16 via PE with diagonal-column lhsT so batch b lands on
    PSUM partition b
  - masked exp + per-block partial row sums run inside the loop (logits are
    bounded by sum|v_w| ~ 23 so exp needs no max subtraction; masked lanes
    get exp(-1e5) = 0); the serial tail is just sum+reciprocal+scale
"""
from contextlib import ExitStack

import numpy as np

B, S, H, E = 32, 2048, 512, 1024
NCORES = 8
B_LOC = B // NCORES
SBLK = 512
WSCALE = 16.0

_CACHE = {}


def _build_kernel(repeats=1, psbufs=2, trbufs=8, natbufs=6, enbufs=4, stage=4, tmode="pe", lmode="bf16", dppbufs=2):
    """stage: 1=loads only, 2=+transposes, 3=+proj matmuls, 4=full.
    tmode: "xbar" = DMA xbar transposes, "pe" = TensorE transposes of u16 pairs."""
    import concourse.tile as tile
    from concourse import bacc, mybir

    F32 = mybir.dt.float32
    BF16 = mybir.dt.bfloat16
    FP8 = mybir.dt.float8e4
    U16 = mybir.dt.uint16
    I32 = mybir.dt.int32
    AF = mybir.ActivationFunctionType
    DR = mybir.MatmulPerfMode.DoubleRow

    NSB = S // SBLK
    NSUB = SBLK // 128
    NEC = E // 128
    EPB = E // 256  # e pair-blocks (256 e's contracted per DoubleRow matmul)
    NHC = H // 128
    NDC = H // 128

    nc = bacc.Bacc("TRN2", target_bir_lowering=False, debug=False, num_devices=NCORES)

    dec = nc.dram_tensor("decoder_hide", [B_LOC, H], F32, kind="ExternalInput")
    enc = nc.dram_tensor("encoder_out", [B_LOC, S, E], F32, kind="ExternalInput")
    msk = nc.dram_tensor("mask", [B_LOC, S], I32, kind="ExternalInput")
    w_attn = nc.dram_tensor("W_attn", [3 * H, H], F32, kind="ExternalInput")
    b_attn = nc.dram_tensor("b_attn", [H], F32, kind="ExternalInput")
    v_w = nc.dram_tensor("v_w", [H], F32, kind="ExternalInput")
    out = nc.dram_tensor("out", [B_LOC, S], F32, kind="ExternalOutput")

    with ExitStack() as ctx:
        tc = ctx.enter_context(tile.TileContext(nc))
        singles = ctx.enter_context(tc.tile_pool(name="singles", bufs=1))
        natp = ctx.enter_context(tc.tile_pool(name="natp", bufs=natbufs))
        trp = ctx.enter_context(tc.tile_pool(name="trp", bufs=trbufs))
        enp = ctx.enter_context(tc.tile_pool(name="enp", bufs=enbufs))
        if tmode == "pe":
            psbufs = min(psbufs, 4)
        n8p = ctx.enter_context(tc.tile_pool(name="n8p", bufs=natbufs))
        psp = ctx.enter_context(tc.tile_pool(name="psp", bufs=psbufs, space="PSUM"))
        attp = ctx.enter_context(tc.tile_pool(name="attp", bufs=2, space="PSUM"))
        dpp = ctx.enter_context(
            tc.tile_pool(name="dpp", bufs=dppbufs, space="PSUM")
        )

        # ---- constants ----
        # W_e in DoubleRow interleave: wdr[p, pb, i, h] = WSCALE * W_e[256*pb + 2*p + i, h]
        wf = singles.tile([128, EPB, 2, H], F32)
        nc.gpsimd.dma_start(
            out=wf[:], in_=w_attn[H:, :].rearrange("(pb p i) h -> p pb i h", p=128, i=2)
        )
        wdr = singles.tile([128, EPB, 2, H], FP8)
        nc.vector.tensor_scalar_mul(wdr[:], wf[:], WSCALE)
        wh_f = singles.tile([128, NDC, H], F32)
        nc.sync.dma_start(
            out=wh_f[:], in_=w_attn[:H, :].rearrange("(dc p) h -> p dc h", p=128)
        )
        batt = singles.tile([128, NHC], F32)
        nc.sync.dma_start(out=batt[:], in_=b_attn.rearrange("(hc p) -> p hc", p=128))
        vw_bf = singles.tile([128, NHC], BF16)
        nc.gpsimd.dma_start(out=vw_bf[:], in_=v_w.rearrange("(hc p) -> p hc", p=128))
        dect = singles.tile([128, NDC, B_LOC], F32)
        for dc in range(NDC):
            nc.gpsimd.dma_start(
                out=dect[:, dc, :],
                in_=dec[:, dc * 128 : (dc + 1) * 128].rearrange("b p -> p b"),
            )
        maskt = singles.tile([B_LOC, S], I32)
        nc.sync.dma_start(out=maskt[:], in_=msk[:, :])

        identu = singles.tile([128, 128], BF16)
        if tmode == "pe":
            from concourse import masks

            masks.make_identity(nc, identu[:])

        # v_w diagonal-column tiles: vz[:, b, hc, :] = [128, B_LOC], col b = v_w chunk hc
        vz = singles.tile([128, B_LOC, NHC, B_LOC], BF16)
        nc.vector.memset(vz[:], 0.0)
        for b in range(B_LOC):
            for hc in range(NHC):
                nc.vector.tensor_copy(vz[:, b, hc, b : b + 1], vw_bf[:, hc : hc + 1])

        # ---- dec_proj bias: decb[:, hc, b] = W_h.T @ dec.T + b_attn ----
        decb = singles.tile([128, NHC, B_LOC], F32)
        for hc in range(NHC):
            dp = dpp.tile([128, B_LOC], F32)
            for dc in range(NDC):
                nc.tensor.matmul(
                    dp[:],
                    wh_f[:, dc, hc * 128 : (hc + 1) * 128],
                    dect[:, dc, :],
                    start=(dc == 0),
                    stop=(dc == NDC - 1),
                )
            nc.scalar.activation(
                decb[:, hc, :], dp[:], AF.Identity, bias=batt[:, hc : hc + 1]
            )

        L = singles.tile([B_LOC, S], F32)
        Ex = singles.tile([B_LOC, S], F32)
        Ssum4 = singles.tile([B_LOC, NSB], F32)

        # ---- main loop over s-blocks ----
        import contextlib

        # For_i places an InstAllEngineBarrier in its per-iteration semaphore
        # reset block -- a full 5-engine drain+refill per trip. The real
        # single-pass kernel has no such barrier, so the timing builds unroll
        # 4 passes per trip to keep the loop overhead out of the slope.
        unroll = 4 if repeats > 1 and repeats % 4 == 0 else 1
        loop_ctx = (
            tc.For_i(0, repeats // unroll, 1) if repeats > 1 else contextlib.nullcontext()
        )
        with loop_ctx:
         for _u in range(unroll):
          for sb in range(NSB):
            if stage >= 4:
                attps = attp.tile([B_LOC, SBLK], F32)
            else:
                attps = None
            pending = []  # lagged v_w matmuls: (b, hc, en)
            for b in range(B_LOC):
                if lmode == "f32":
                    natf = natp.tile([128, NSUB, E], F32, name="natf")
                    nc.sync.dma_start(
                        out=natf[:],
                        in_=enc[b, sb * SBLK : (sb + 1) * SBLK, :].rearrange(
                            "(sub p) e -> p sub e", p=128
                        ),
                    )
                else:
                    natf = natp.tile([128, NSUB, E], BF16, name="natf")
                    nc.gpsimd.dma_start(
                        out=natf[:],
                        in_=enc[b, sb * SBLK : (sb + 1) * SBLK, :].rearrange(
                            "(sub p) e -> p sub e", p=128
                        ),
                    )
                # round-to-nearest f32->fp8 (the DMA's direct cast truncates,
                # which doubles the quantization noise); alternate DVE/ACT so
                # neither engine owns the whole 32MB conversion
                nat = n8p.tile([128, NSUB, E], FP8)
                nc.vector.tensor_copy(nat[:], natf[:])
                if stage < 2:
                    nc.vector.tensor_copy(
                        L[0:1, sb * SBLK + b : sb * SBLK + b + 1], nat[0:1, 0, 0:1]
                    )
                    continue
                # tpr[p, pb, sub, s0] = u16 pair (e=2*(pb*128+p), e+1) at s = sub*128+s0
                tpr = trp.tile([128, EPB, NSUB, 128], U16)
                nat16 = nat[:].bitcast(U16)
                if tmode == "xbar":
                    for sub in range(NSUB):
                        eng = nc.sync if b % 2 == 0 else nc.scalar
                        eng.dma_start(
                            out=tpr[:, :, sub, :], in_=nat16[:, sub, :], transpose=True
                        )
                else:
                    # the u16 pair data rides through the PE transpose as bf16
                    # (bit-identical view; Ldweights rejects integer dtypes)
                    for pb in range(EPB):
                        trps = dpp.tile([128, NSUB * 128], BF16)
                        for sub in range(NSUB):
                            nc.tensor.transpose(
                                trps[:, sub * 128 : (sub + 1) * 128],
                                nat16[:, sub, pb * 128 : (pb + 1) * 128].bitcast(BF16),
                                identu[:],
                            )
                        nc.vector.tensor_copy(tpr[:, pb, :, :], trps[:].bitcast(U16))
                if stage < 3:
                    nc.vector.tensor_copy(
                        L[0:1, sb * SBLK + b : sb * SBLK + b + 1], tpr[0:1, 0, 0, 0:1]
                    )
                    continue
                for hc in range(NHC):
                    ps = psp.tile([128, SBLK], F32)
                    for ecp in range(EPB):
                        rhs = (
                            tpr[:, ecp, :, :]
                            .bitcast(FP8)
                            .rearrange("p sub (s i) -> p i (sub s)", i=2)
                        )
                        nc.tensor.matmul(
                            ps[:],
                            wdr[:, ecp, :, hc * 128 : (hc + 1) * 128],
                            rhs,
                            start=(ecp == 0),
                            stop=(ecp == EPB - 1),
                            perf_mode=DR,
                        )
                    if stage < 4:
                        nc.vector.tensor_copy(
                            L[0:1, sb * SBLK + b * NHC + hc : sb * SBLK + b * NHC + hc + 1],
                            ps[0:1, 0:1],
                        )
                        continue
                    en = enp.tile([128, SBLK], BF16)
                    nc.scalar.activation(
                        en[:],
                        ps[:],
                        AF.Tanh,
                        bias=decb[:, hc, b : b + 1],
                        scale=1.0 / WSCALE,
                    )
                    pending.append((b, hc, en))
                    if len(pending) > 2:
                        pb, phc, pen = pending.pop(0)
                        nc.tensor.matmul(
                            attps[:],
                            vz[:, pb, phc, :],
                            pen[:],
                            start=(pb == 0 and phc == 0),
                            stop=(pb == B_LOC - 1 and phc == NHC - 1),
                        )
            if stage >= 4:
                for pb, phc, pen in pending:
                    nc.tensor.matmul(
                        attps[:],
                        vz[:, pb, phc, :],
                        pen[:],
                        start=(pb == 0 and phc == 0),
                        stop=(pb == B_LOC - 1 and phc == NHC - 1),
                    )
                # masked exp in-loop: logits are bounded (|att| <= sum|v_w| ~ 23),
                # so exp needs no max subtraction; masked lanes get exp(-1e5)=0.
                # accum_out gives this s-block's partial row sums.
                Lms = singles.tile([B_LOC, SBLK], F32, name=f"Lms{sb % 2}")
                nc.vector.memset(Lms[:], -100000.0)
                nc.vector.copy_predicated(
                    Lms[:], maskt[:, sb * SBLK : (sb + 1) * SBLK], attps[:]
                )
                nc.scalar.activation(
                    Ex[:, sb * SBLK : (sb + 1) * SBLK],
                    Lms[:],
                    AF.Exp,
                    accum_out=Ssum4[:, sb : sb + 1],
                )

        # ---- finish softmax: total sums -> reciprocal -> scale -> store ----
        Ssum = singles.tile([B_LOC, 1], F32)
        nc.vector.tensor_reduce(
            Ssum[:], Ssum4[:], axis=mybir.AxisListType.X, op=mybir.AluOpType.add
        )
        R = singles.tile([B_LOC, 1], F32)
        nc.vector.reciprocal(R[:], Ssum[:])
        O = singles.tile([B_LOC, S], F32)
        nc.vector.tensor_scalar_mul(O[:], Ex[:], R[:])
        nc.sync.dma_start(out=out[:, :], in_=O[:])

    nc.compile()
    return nc


def _get_state():
    if _CACHE:
        return _CACHE
    import jax
    from jax.experimental.shard_map import shard_map
    from jax.sharding import Mesh, PartitionSpec
    from concourse import bass2jax, mybir

    nc = _build_kernel()
    bass2jax.install_neuronx_cc_hook()

    partition_name = nc.partition_id_tensor.name if nc.partition_id_tensor else None
    in_names: list[str] = []
    out_names: list[str] = []
    out_avals = []
    zero_shapes = []
    for alloc in nc.m.functions[0].allocations:
        if not isinstance(alloc, mybir.MemoryLocationSet):
            continue
        name = alloc.memorylocations[0].name
        if alloc.kind == "ExternalInput":
            if name != partition_name:
                in_names.append(name)
        elif alloc.kind == "ExternalOutput":
            shape = tuple(alloc.tensor_shape)
            dtype = mybir.dt.np(alloc.dtype)
            out_names.append(name)
            out_avals.append(jax.core.ShapedArray(shape, dtype))
            zero_shapes.append((shape, dtype))
    n_params = len(in_names)
    all_names = list(in_names + out_names)
    if partition_name is not None:
        all_names.append(partition_name)
    all_names = tuple(all_names)

    def _body(*args):
        operands = list(args)
        if partition_name is not None:
            operands.append(bass2jax.partition_id_tensor())
        outs = bass2jax._bass_exec_p.bind(
            *operands,
            out_avals=tuple(out_avals),
            in_names=all_names,
            out_names=tuple(out_names),
            lowering_input_output_aliases=(),
            sim_require_finite=True,
            sim_require_nnan=True,
            nc=nc,
        )
        return tuple(outs)

    devices = jax.devices()[:NCORES]
    mesh = Mesh(np.asarray(devices), ("core",))
    n_outs = len(out_names)
    in_specs = (PartitionSpec("core"),) * (n_params + n_outs)
    out_specs = (PartitionSpec("core"),) * n_outs
    donate = tuple(range(n_params, n_params + n_outs))
    fn = jax.jit(
        shard_map(_body, mesh=mesh, in_specs=in_specs, out_specs=out_specs, check_rep=False),
        donate_argnums=donate,
        keep_unused=True,
    )
    _CACHE.update(
        dict(fn=fn, nc=nc, in_names=in_names, out_names=out_names, zero_shapes=zero_shapes, mesh=mesh)
    )
    return _CACHE


def _concat_inputs(inputs):
    """Build the global (concat over cores on axis 0) arrays in in_names order."""
    st = _get_state()
    per_name = {}
    # per-core shards
    dec_s = inputs["decoder_hide"].reshape(NCORES, B_LOC, H)
    enc_s = inputs["encoder_out"].reshape(NCORES, B_LOC, S, E)
    msk_s = inputs["mask"].reshape(NCORES, B_LOC, S)
    per_name["decoder_hide"] = dec_s.reshape(NCORES * B_LOC, H)
    per_name["encoder_out"] = enc_s.reshape(NCORES * B_LOC, S, E)
    per_name["mask"] = msk_s.reshape(NCORES * B_LOC, S)
    # replicated weights: tile along axis 0
    per_name["W_attn"] = np.tile(inputs["W_attn"], (NCORES, 1))
    per_name["b_attn"] = np.tile(inputs["b_attn"], NCORES)
    per_name["v_w"] = np.tile(inputs["v_w"], NCORES)
    return [np.ascontiguousarray(per_name[n]) for n in st["in_names"]]


def _zero_outs():
    st = _get_state()
    return [
        np.zeros((NCORES * shape[0], *shape[1:]), dtype) for shape, dtype in st["zero_shapes"]
    ]


def kernel(**inputs) -> np.ndarray:
    st = _get_state()
    concat_in = _concat_inputs(inputs)
    outs = st["fn"](*concat_in, *_zero_outs())
    out = np.asarray(outs[st["out_names"].index("out")])
    return out.reshape(B, S)

